# revision 7
# baseline (speedup 1.0000x reference)
"""Trainium2 Bass kernel for nn_DTFN_38405597561803 (gnn_message_passing).

Model (reference):
    h  = emb[x]                                   # [B,S,D] gather
    m  = softplus(h @ w_mass.T + b_mass) + EPS
    dt = sigmoid(cfl_raw)
    repeat K=3:
        hid = tanh(left @ w_f1_l.T + right @ w_f1_r.T + b_f1)
        F   = softplus(hid @ w_f2.T + b_f2)
        m   = max(m + dt * (F[i-1] - F[i]), EPS)              # 1-D flux stencil
    out = m @ w_dec.T + b_dec                      # [B,S,V] decode (memory bound)

Sharding: sequence-parallel, 8 cores = 4 batches x 2 halves of S=2048, with a
K=3 halo built host-side (no device-to-device traffic).  The decode write
stream (131 MB/core) is the HBM roofline term (~333 us at the measured
~394 GB/s/core), so the whole point of the schedule is to start that stream
as early as possible and never let it stall:

  * The core's 1024 owned positions are split into 4 quarters.  Each quarter
    runs the K-step stencil in a private 262-wide buffer with a 3-position
    halo on each side (the same overlap trick used between cores, applied
    within a core) - exact, validated vs reference in fp64/fp32.
  * Emission order: gather -> mass(chunk0) -> stencil(q0) -> mass(1,2) ->
    decode(q0) interleaved 1-op-per-vocab-block with stencil(q1) -> ... so
    the first output DMA issues ~15 us in, and stencils for later quarters
    hide under the ~84 us decode stream of the previous quarter.
  * w_dec is converted to bf16 on the host (halves its HBM read to 8.2 MB)
    and streamed on the SP ring during the prologue; the decode matmul runs
    bf16 x bf16 (tolerance 2e-2 >> bf16 error here).
  * Decode per tile: K=1 ones x bias matmul initializes PSUM with b_dec,
    then the bf16 m-tile x w_dec.T matmul accumulates; PSUM->SBUF copies
    alternate Scalar/Vector engines; SBUF->HBM writes alternate SP/ACT
    HWDGE rings.
"""

import sys

if "/opt/trn_rl_repo" not in sys.path:
    sys.path.insert(0, "/opt/trn_rl_repo")

import numpy as np

B, S, D, V, K = 4, 2048, 128, 32000, 3
EPS = 1e-6
NCORES = 8
HALO = K                      # 3
LOWN = S // 2                 # 1024 owned positions per core
L = LOWN + 2 * HALO           # 1030 local positions
NT = 9                        # gather tiles (covers 1152 >= 1032)
LPAD = NT * 128               # 1152
NQ = 4                        # stencil quarters per core
QOWN = LOWN // NQ             # 256 owned positions per quarter
QW = QOWN + 2 * HALO          # 262 extended stencil width (even, >=256)
QWP = QW + 2                  # 264 private buffer width (2 finite pad cols)
VBLK = 500                    # vocab block (<= 512 PSUM bank, 64*500 = 32000)
NVB = V // VBLK               # 64
RT = LOWN // 128              # 8 decode row tiles per core
WDEC_CHUNK = 500              # w_decT load granularity on the SP ring
MASS_CHUNKS = ((0, 266), (266, 512), (778, 254))   # covers mT[0:1032]

_CACHE = {}


def build_program(decode_reps=1):
    """Build (and bacc-compile) the single-core SPMD Bass program.

    decode_reps > 1 wraps the whole per-pass body (gather+mass+stencil+decode)
    in a hardware For_i loop - benchmarking only, to measure the steady-state
    pass time by slope.
    """
    import concourse.bacc as bacc
    import concourse.bass as bass
    import concourse.tile as tile
    from concourse import mybir

    f32 = mybir.dt.float32
    f32r = mybir.dt.float32r
    bf16 = mybir.dt.bfloat16
    i32 = mybir.dt.int32
    AF = mybir.ActivationFunctionType

    nc = bacc.Bacc(
        trn_type="TRN2",
        target_bir_lowering=False,
        debug=False,
        enable_asserts=False,
        num_devices=NCORES,
    )

    d_idx = nc.dram_tensor("idx", [128, NT], i32, kind="ExternalInput").ap()
    d_emb = nc.dram_tensor("emb", [V, D], f32, kind="ExternalInput").ap()
    d_wm = nc.dram_tensor("wmT", [D, D], f32r, kind="ExternalInput").ap()
    d_wl = nc.dram_tensor("wf1lT", [D, D], f32r, kind="ExternalInput").ap()
    d_wr = nc.dram_tensor("wf1rT", [D, D], f32r, kind="ExternalInput").ap()
    d_w2 = nc.dram_tensor("wf2T", [D, D], f32r, kind="ExternalInput").ap()
    d_bm = nc.dram_tensor("b_mass", [D, 1], f32, kind="ExternalInput").ap()
    d_b1 = nc.dram_tensor("b_f1", [D, 1], f32, kind="ExternalInput").ap()
    d_b2 = nc.dram_tensor("b_f2", [D, 1], f32, kind="ExternalInput").ap()
    d_mask = nc.dram_tensor("maskq", [D, NQ * QW], f32, kind="ExternalInput").ap()
    d_ones = nc.dram_tensor("ones16", [1, D], bf16, kind="ExternalInput").ap()
    d_bdec = nc.dram_tensor("bdec16", [1, V], bf16, kind="ExternalInput").ap()
    d_wdec = nc.dram_tensor("wdec16T", [D, V], bf16, kind="ExternalInput").ap()
    d_ident = nc.dram_tensor("ident", [D, D], f32, kind="ExternalInput").ap()
    d_out = nc.dram_tensor("out", [LOWN, V], f32, kind="ExternalOutput").ap()

    def r(ap):
        return ap.bitcast(f32r)

    with tile.TileContext(nc) as tc:
        with tc.tile_pool(name="const", bufs=1) as const:
            wdec_sb = const.tile([D, V], bf16)
            wm_sb = const.tile([D, D], f32r)
            wl_sb = const.tile([D, D], f32r)
            wr_sb = const.tile([D, D], f32r)
            w2_sb = const.tile([D, D], f32r)
            bm_sb = const.tile([D, 1], f32)
            b1_sb = const.tile([D, 1], f32)
            b2_sb = const.tile([D, 1], f32)
            mask_sb = const.tile([D, NQ * QW], f32)
            ones_sb = const.tile([1, D], bf16)
            bdec_sb = const.tile([1, V], bf16)
            ident_sb = const.tile([D, D], f32)
            it = const.tile([128, NT], i32)
            warm = const.tile([128, 2], f32)

            hT = const.tile([D, LPAD], f32)
            mT = const.tile([D, L + 2], f32)     # 2 finite pad cols
            m16 = const.tile([D, LOWN], bf16)    # final mass, decode operand
            mq = const.tile([D, QWP], f32)       # private quarter stencil buf
            fq = const.tile([D, QWP], f32)       # col 0 = zero border
            dmq = const.tile([D, QW], f32)

            # preload the ACT function table while DMAs stream
            nc.vector.memset(warm[:], 0.0)
            nc.scalar.activation(warm[:], warm[:], AF.Exp)
            nc.vector.memset(fq[:], 0.0)

            nc.scalar.dma_start(it[:], d_idx[:])
            nc.scalar.dma_start(wm_sb[:], d_wm[:])
            nc.scalar.dma_start(wl_sb[:], d_wl[:])
            nc.scalar.dma_start(wr_sb[:], d_wr[:])
            nc.scalar.dma_start(w2_sb[:], d_w2[:])
            nc.scalar.dma_start(bm_sb[:], d_bm[:])
            nc.scalar.dma_start(b1_sb[:], d_b1[:])
            nc.scalar.dma_start(b2_sb[:], d_b2[:])
            nc.scalar.dma_start(mask_sb[:], d_mask[:])
            nc.scalar.dma_start(ones_sb[:], d_ones[:])
            nc.scalar.dma_start(bdec_sb[:], d_bdec[:])
            nc.scalar.dma_start(ident_sb[:], d_ident[:])

            # w_dec.T (bf16) streams on the SP ring; decode consumes blocks
            # in the same order, so it only ever waits for the block in hand.
            for i in range(V // WDEC_CHUNK):
                sl = slice(i * WDEC_CHUNK, (i + 1) * WDEC_CHUNK)
                nc.sync.dma_start(wdec_sb[:, sl], d_wdec[:, sl])

            from contextlib import nullcontext
            with (
                tc.tile_pool(name="gpool", bufs=9) as gpool,
                tc.tile_pool(name="tpsum", bufs=1, space="PSUM") as tpsum,
                tc.tile_pool(name="mpsum", bufs=2, space="PSUM") as mpsum,
                tc.tile_pool(name="dpsum", bufs=5, space="PSUM") as dpsum,
                tc.tile_pool(name="stage", bufs=12) as stage,
                tc.tile_pool(name="hidp", bufs=2) as hidp,
                tc.tile_pool(name="biasp", bufs=4) as biasp,
                tc.For_i(
                    0, decode_reps, 1,
                    hint_engines=(
                        mybir.EngineType.PE, mybir.EngineType.Activation,
                        mybir.EngineType.DVE, mybir.EngineType.SP,
                        mybir.EngineType.Pool,
                    ),
                ) if decode_reps > 1 else nullcontext(),
            ):
                # ---- embed gather -> transpose -> hT [D, positions] ----
                for t in range(NT):
                    g = gpool.tile([128, D], f32)
                    nc.gpsimd.indirect_dma_start(
                        out=g[:],
                        out_offset=None,
                        in_=d_emb[:, :],
                        in_offset=bass.IndirectOffsetOnAxis(ap=it[:, t : t + 1], axis=0),
                    )
                    pt = tpsum.tile([128, 128], f32)
                    nc.tensor.transpose(pt[:], g[:], ident_sb[:])
                    nc.vector.tensor_copy(r(hT[:, t * 128 : (t + 1) * 128]), pt[:])

                # ---- mass layer: mT = softplus(wmT.T @ hT + b_mass) + EPS ----
                # softplus(z) = ln(exp(z) + 1)
                def mass_chunk(off, n):
                    pm = mpsum.tile([128, 512], f32, name="ps")
                    nc.tensor.matmul(
                        pm[:, :n], lhsT=wm_sb[:], rhs=r(hT[:, off : off + n]),
                        start=True, stop=True,
                    )
                    nc.scalar.activation(pm[:, :n], pm[:, :n], AF.Exp, bias=bm_sb[:, :1])
                    nc.scalar.activation(r(mT[:, off : off + n]), pm[:, :n], AF.Ln, bias=1.0)
                    nc.vector.tensor_scalar_add(
                        r(mT[:, off : off + n]), mT[:, off : off + n], EPS
                    )

                def stencil_gen(q):
                    """K flux steps for quarter q, yielded in ~engine-balanced
                    units so decode tiles can be interleaved between them."""
                    nc.vector.tensor_copy(r(mq[:]), mT[:, QOWN * q : QOWN * q + QWP])
                    yield
                    for k in range(K):
                        p1 = mpsum.tile([128, 512], f32, name="ps")
                        nc.tensor.matmul(
                            p1[:, :QW], lhsT=wl_sb[:], rhs=r(mq[:, 0:QW]),
                            start=True, stop=False,
                        )
                        nc.tensor.matmul(
                            p1[:, :QW], lhsT=wr_sb[:], rhs=r(mq[:, 1 : 1 + QW]),
                            start=False, stop=True,
                        )
                        yield
                        # tanh(z) = 1 - 2/(exp(2z) + 1); b_f1 pre-doubled host-side
                        nc.scalar.activation(
                            p1[:, :QW], p1[:, :QW], AF.Exp, bias=b1_sb[:, :1], scale=2.0
                        )
                        yield
                        nc.vector.tensor_scalar_add(p1[:, :QW], p1[:, :QW], 1.0)
                        hid = hidp.tile([128, QW], f32)
                        with nc.allow_low_precision(reason="f32r is fp32-width"):
                            nc.vector.reciprocal(r(hid[:]), p1[:, :QW])
                        yield
                        nc.vector.tensor_scalar(
                            r(hid[:]), hid[:], -2.0, 1.0,
                            op0=mybir.AluOpType.mult, op1=mybir.AluOpType.add,
                        )
                        yield
                        p2 = mpsum.tile([128, 512], f32, name="ps")
                        nc.tensor.matmul(
                            p2[:, :QW], lhsT=w2_sb[:], rhs=r(hid[:]),
                            start=True, stop=True,
                        )
                        yield
                        nc.scalar.activation(p2[:, :QW], p2[:, :QW], AF.Exp, bias=b2_sb[:, :1])
                        yield
                        nc.scalar.activation(fq[:, 1 : 1 + QW], p2[:, :QW], AF.Ln, bias=1.0)
                        yield
                        nc.vector.tensor_mul(
                            fq[:, 1 : 1 + QW], fq[:, 1 : 1 + QW],
                            mask_sb[:, q * QW : (q + 1) * QW],
                        )
                        yield
                        nc.vector.tensor_sub(dmq[:], fq[:, 0:QW], fq[:, 1 : 1 + QW])
                        yield
                        nc.vector.tensor_add(r(mq[:, 0:QW]), mq[:, 0:QW], dmq[:])
                        if k < K - 1:
                            nc.vector.tensor_scalar_max(r(mq[:, 0:QW]), mq[:, 0:QW], EPS)
                        else:
                            with nc.allow_low_precision(reason="bf16 decode operand"):
                                nc.vector.tensor_scalar_max(
                                    m16[:, q * QOWN : (q + 1) * QOWN],
                                    mq[:, HALO : HALO + QOWN], EPS,
                                )
                        yield

                mass_chunk(*MASS_CHUNKS[0])
                for _ in stencil_gen(0):      # quarter 0: serial, critical path
                    pass
                mass_chunk(*MASS_CHUNKS[1])
                mass_chunk(*MASS_CHUNKS[2])

                # ---- decode: out[r, v] = m_rows @ w_dec.T + b_dec ----
                # quarter-major; stencil for quarter q+1 interleaves 1 unit
                # per vocab block under quarter q's decode stream.
                i = 0
                for q in range(NQ):
                    gen = stencil_gen(q + 1) if q + 1 < NQ else None
                    for v in range(NVB):
                        vs = slice(v * VBLK, (v + 1) * VBLK)
                        for rt in (2 * q, 2 * q + 1):
                            m_blk = m16[:, rt * 128 : (rt + 1) * 128]
                            pd = dpsum.tile([128, VBLK], f32)
                            nc.tensor.matmul(
                                pd[:], lhsT=ones_sb[:1, :], rhs=bdec_sb[:1, vs],
                                start=True, stop=False,
                            )
                            nc.tensor.matmul(
                                pd[:], lhsT=m_blk, rhs=wdec_sb[:, vs],
                                start=False, stop=True,
                            )
                            st = stage.tile([128, VBLK], f32)
                            if i % 2 == 0:
                                nc.scalar.copy(st[:], pd[:])
                            else:
                                nc.vector.tensor_copy(st[:], pd[:])
                            dma_eng = nc.sync if i % 2 == 0 else nc.scalar
                            dma_eng.dma_start(d_out[rt * 128 : (rt + 1) * 128, vs], st[:])
                            i += 1
                        if gen is not None:
                            next(gen, None)
                    if gen is not None:
                        for _ in gen:
                            pass

    # Keep every ACT function in one table load (Exp, Ln, Copy all live in
    # 'natural_log_exp_and_others'); blank other tables so the table-load
    # insertion pass can't thrash LoadActFuncSet (~1.3us each).
    import concourse.bacc as bacc_mod
    orig_get_tables = bacc_mod.get_activation_tables

    def only_ln_exp(arch):
        tabs = orig_get_tables(arch)
        return {
            k: (v if k == "natural_log_exp_and_others" else set())
            for k, v in tabs.items()
        }

    bacc_mod.get_activation_tables = only_ln_exp
    try:
        nc.compile()
    finally:
        bacc_mod.get_activation_tables = orig_get_tables
    return nc


def _prep_inputs(inputs):
    """Host-side shard prep: per-core input dict list."""
    import ml_dtypes

    bf16 = ml_dtypes.bfloat16

    x = np.asarray(inputs["x"]).astype(np.int32)            # [B, S]
    emb = np.ascontiguousarray(np.asarray(inputs["emb"], np.float32))
    w_mass = np.asarray(inputs["w_mass"], np.float32)
    b_mass = np.asarray(inputs["b_mass"], np.float32)
    w_f1 = np.asarray(inputs["w_f1"], np.float32)
    b_f1 = np.asarray(inputs["b_f1"], np.float32)
    w_f2 = np.asarray(inputs["w_f2"], np.float32)
    b_f2 = np.asarray(inputs["b_f2"], np.float32)
    cfl = float(np.asarray(inputs["cfl_raw"]))
    w_dec = np.asarray(inputs["w_dec"], np.float32)
    b_dec = np.asarray(inputs["b_dec"], np.float32)

    dt = float(1.0 / (1.0 + np.exp(-cfl)))

    common = {
        "emb": emb,
        "wmT": np.ascontiguousarray(w_mass.T),
        "wf1lT": np.ascontiguousarray(w_f1[:, :D].T),
        "wf1rT": np.ascontiguousarray(w_f1[:, D:].T),
        "wf2T": np.ascontiguousarray(w_f2.T),
        "b_mass": np.ascontiguousarray(b_mass[:, None]),
        # device computes tanh(z+b) as 1 - 2/(exp(2z + 2b) + 1) with scale=2 on z
        "b_f1": np.ascontiguousarray((2.0 * b_f1)[:, None]),
        "b_f2": np.ascontiguousarray(b_f2[:, None]),
        "ones16": np.ones((1, D), bf16),
        "bdec16": np.ascontiguousarray(b_dec.astype(bf16)[None, :]),
        "wdec16T": np.ascontiguousarray(w_dec.T.astype(bf16)),
        "ident": np.eye(D, dtype=np.float32),
    }

    in_maps = []
    for c in range(NCORES):
        b, half = divmod(c, 2)
        idx = np.zeros(LPAD, np.int32)
        if half == 0:
            idx[HALO : HALO + (L - HALO)] = x[b, 0 : L - HALO]
        else:
            idx[0 : L - HALO] = x[b, S - (L - HALO) : S]
        # per-quarter edge masks: dt everywhere; fake edge (QW-1) always 0;
        # true-boundary sides zero the outer 3 edges (halo-overlap trick
        # handles interior quarter boundaries with no masking).
        maskq = np.full((NQ, QW), dt, np.float32)
        maskq[:, QW - 1] = 0.0
        if half == 0:
            maskq[0, 0:HALO] = 0.0
        else:
            maskq[NQ - 1, QW - 1 - HALO : QW - 1] = 0.0
        m = dict(common)
        m["idx"] = np.ascontiguousarray(idx.reshape(NT, 128).T)     # [128, NT]
        m["maskq"] = np.ascontiguousarray(
            np.broadcast_to(maskq.reshape(1, NQ * QW), (D, NQ * QW)).astype(np.float32)
        )
        in_maps.append(m)
    return in_maps


def get_program():
    if "nc" not in _CACHE:
        _CACHE["nc"] = build_program()
    return _CACHE["nc"]


def run(inputs, trace=False, **kw):
    """Returns (full_output [B,S,V] float32, BassKernelResults)."""
    from concourse.bass_utils import run_bass_kernel_spmd

    nc = get_program()
    in_maps = _prep_inputs(inputs)
    res = run_bass_kernel_spmd(
        nc, in_maps, core_ids=list(range(NCORES)), trace=trace, **kw
    )
    full = np.empty((B * S, V), np.float32)
    for c in range(NCORES):
        full[c * LOWN : (c + 1) * LOWN] = res.results[c]["out"]
    return full.reshape(B, S, V), res


def kernel(**inputs):
    out, _ = run(inputs, trace=False)
    return out


# revision 16
# speedup vs baseline: 4.9449x; 4.9449x over previous
"""Trainium2 Bass kernel for nn_DTFN_38405597561803 (gnn_message_passing).

Model (reference):
    h  = emb[x]                                   # [B,S,D] gather
    m  = softplus(h @ w_mass.T + b_mass) + EPS
    dt = sigmoid(cfl_raw)
    repeat K=3:
        hid = tanh(left @ w_f1_l.T + right @ w_f1_r.T + b_f1)
        F   = softplus(hid @ w_f2.T + b_f2)
        m   = max(m + dt * (F[i-1] - F[i]), EPS)              # 1-D flux stencil
    out = m @ w_dec.T + b_dec                      # [B,S,V] decode (memory bound)

Sharding: sequence-parallel, 8 cores = 4 batches x 2 halves of S=2048, with a
K=3 halo built host-side (no device-to-device traffic).  The decode write
stream (131 MB/core) is the HBM roofline term (~333 us at the measured
~394 GB/s/core), so the whole point of the schedule is to start that stream
as early as possible and never let it stall:

  * The core's 1024 owned positions are split into 4 quarters.  Each quarter
    runs the K-step stencil in a private 262-wide buffer with a 3-position
    halo on each side (the same overlap trick used between cores, applied
    within a core) - exact, validated vs reference in fp64/fp32.
  * Emission order: gather -> mass(chunk0) -> stencil(q0) -> mass(1,2) ->
    decode(q0) interleaved 1-op-per-vocab-block with stencil(q1) -> ... so
    the first output DMA issues ~15 us in, and stencils for later quarters
    hide under the ~84 us decode stream of the previous quarter.
  * w_dec is converted to bf16 on the host (halves its HBM read to 8.2 MB)
    and streamed on the SP ring during the prologue; the decode matmul runs
    bf16 x bf16 (tolerance 2e-2 >> bf16 error here).
  * Decode per tile: K=1 ones x bias matmul initializes PSUM with b_dec,
    then the bf16 m-tile x w_dec.T matmul accumulates; PSUM->SBUF copies
    alternate Scalar/Vector engines; SBUF->HBM writes alternate SP/ACT
    HWDGE rings.
"""

import sys

if "/opt/trn_rl_repo" not in sys.path:
    sys.path.insert(0, "/opt/trn_rl_repo")

import numpy as np

B, S, D, V, K = 4, 2048, 128, 32000, 3
EPS = 1e-6
NCORES = 8
HALO = K                      # 3
LOWN = S // 2                 # 1024 owned positions per core
L = LOWN + 2 * HALO           # 1030 local positions
NT = 9                        # gather tiles (covers 1152 >= 1032)
LPAD = NT * 128               # 1152
NQ = 4                        # stencil quarters per core
QOWN = LOWN // NQ             # 256 owned positions per quarter
QW = QOWN + 2 * HALO          # 262 extended stencil width (even, >=256)
QWP = QW + 2                  # 264 private buffer width (2 finite pad cols)
VBLK = 500                    # vocab block (<= 512 PSUM bank, 64*500 = 32000)
NVB = V // VBLK               # 64
RT = LOWN // 128              # 8 decode row tiles per core
WDEC_CHUNK = 500              # w_decT load granularity on the SP ring
MASS_CHUNKS = ((0, 266), (266, 512), (778, 254))   # covers mT[0:1032]

_CACHE = {}


def build_program(decode_reps=1, interleave=True, stencil_eng="gpsimd", grp=2,
                  rings=("sync", "scalar")):
    """Build (and bacc-compile) the single-core SPMD Bass program.

    decode_reps > 1 wraps the whole per-pass body (gather+mass+stencil+decode)
    in a hardware For_i loop - benchmarking only, to measure the steady-state
    pass time by slope.
    """
    import concourse.bacc as bacc
    import concourse.bass as bass
    import concourse.tile as tile
    from concourse import mybir

    f32 = mybir.dt.float32
    f32r = mybir.dt.float32r
    bf16 = mybir.dt.bfloat16
    i32 = mybir.dt.int32
    AF = mybir.ActivationFunctionType

    nc = bacc.Bacc(
        trn_type="TRN2",
        target_bir_lowering=False,
        debug=False,
        enable_asserts=False,
        num_devices=NCORES,
    )

    d_idx = nc.dram_tensor("idx", [128, NT], i32, kind="ExternalInput").ap()
    d_emb = nc.dram_tensor("emb", [V, D], f32, kind="ExternalInput").ap()
    d_wm = nc.dram_tensor("wmT", [D, D], f32r, kind="ExternalInput").ap()
    d_wl = nc.dram_tensor("wf1lT", [D, D], f32r, kind="ExternalInput").ap()
    d_wr = nc.dram_tensor("wf1rT", [D, D], f32r, kind="ExternalInput").ap()
    d_w2 = nc.dram_tensor("wf2T", [D, D], f32r, kind="ExternalInput").ap()
    d_bm = nc.dram_tensor("b_mass", [D, 1], f32, kind="ExternalInput").ap()
    d_b1 = nc.dram_tensor("b_f1", [D, 1], f32, kind="ExternalInput").ap()
    d_b2 = nc.dram_tensor("b_f2", [D, 1], f32, kind="ExternalInput").ap()
    d_mask = nc.dram_tensor("maskq", [D, NQ * QW], f32, kind="ExternalInput").ap()
    d_ones = nc.dram_tensor("ones16", [1, D], bf16, kind="ExternalInput").ap()
    d_bdec = nc.dram_tensor("bdec16", [1, V], bf16, kind="ExternalInput").ap()
    d_wdec = nc.dram_tensor("wdec16T", [D, V], bf16, kind="ExternalInput").ap()
    d_ident = nc.dram_tensor("ident", [D, D], f32, kind="ExternalInput").ap()
    d_out = nc.dram_tensor("out", [LOWN, V], f32, kind="ExternalOutput").ap()

    def r(ap):
        return ap.bitcast(f32r)

    with tile.TileContext(nc) as tc:
        with tc.tile_pool(name="const", bufs=1) as const:
            wdec_sb = const.tile([D, V], bf16)
            wm_sb = const.tile([D, D], f32r)
            wl_sb = const.tile([D, D], f32r)
            wr_sb = const.tile([D, D], f32r)
            w2_sb = const.tile([D, D], f32r)
            bm_sb = const.tile([D, 1], f32)
            b1_sb = const.tile([D, 1], f32)
            b2_sb = const.tile([D, 1], f32)
            mask_sb = const.tile([D, NQ * QW], f32)
            ones_sb = const.tile([1, D], bf16)
            bdec_sb = const.tile([1, V], bf16)
            ident_sb = const.tile([D, D], f32)
            it = const.tile([128, NT], i32)
            warm = const.tile([128, 2], f32)

            hT = const.tile([D, LPAD], f32)
            mT = const.tile([D, L + 2], f32)     # 2 finite pad cols
            m16 = const.tile([D, LOWN], bf16)    # final mass, decode operand
            # double-buffered so adjacent quarters' stencils can pipeline
            mqs = [const.tile([D, QWP], f32, name=f"mq{j}") for j in range(2)]
            fqs = [const.tile([D, QWP], f32, name=f"fq{j}") for j in range(2)]
            dmqs = [const.tile([D, QW], f32, name=f"dmq{j}") for j in range(2)]

            # preload the ACT function table while DMAs stream
            nc.vector.memset(warm[:], 0.0)
            nc.scalar.activation(warm[:], warm[:], AF.Exp)
            nc.vector.memset(fqs[0][:], 0.0)
            nc.vector.memset(fqs[1][:], 0.0)

            nc.scalar.dma_start(it[:], d_idx[:])
            nc.scalar.dma_start(wm_sb[:], d_wm[:])
            nc.scalar.dma_start(wl_sb[:], d_wl[:])
            nc.scalar.dma_start(wr_sb[:], d_wr[:])
            nc.scalar.dma_start(w2_sb[:], d_w2[:])
            nc.scalar.dma_start(bm_sb[:], d_bm[:])
            nc.scalar.dma_start(b1_sb[:], d_b1[:])
            nc.scalar.dma_start(b2_sb[:], d_b2[:])
            nc.scalar.dma_start(mask_sb[:], d_mask[:])
            nc.scalar.dma_start(ones_sb[:], d_ones[:])
            nc.scalar.dma_start(bdec_sb[:], d_bdec[:])
            nc.scalar.dma_start(ident_sb[:], d_ident[:])

            # w_dec.T (bf16) streams on the SP ring; decode consumes blocks
            # in the same order, so it only ever waits for the block in hand.
            for i in range(V // WDEC_CHUNK):
                sl = slice(i * WDEC_CHUNK, (i + 1) * WDEC_CHUNK)
                nc.sync.dma_start(wdec_sb[:, sl], d_wdec[:, sl])

            from contextlib import nullcontext
            with (
                tc.tile_pool(name="gpool", bufs=9) as gpool,
                tc.tile_pool(name="tpsum", bufs=1, space="PSUM") as tpsum,
                tc.tile_pool(name="mpsum", bufs=2, space="PSUM") as mpsum,
                tc.tile_pool(name="dpsum", bufs=5, space="PSUM") as dpsum,
                tc.tile_pool(name="stage", bufs={1: 12, 2: 8, 4: 5}[grp]) as stage,
                tc.tile_pool(name="hidp", bufs=2) as hidp,
                tc.tile_pool(name="biasp", bufs=4) as biasp,
                tc.For_i(
                    0, decode_reps, 1,
                    hint_engines=(
                        mybir.EngineType.PE, mybir.EngineType.Activation,
                        mybir.EngineType.DVE, mybir.EngineType.SP,
                        mybir.EngineType.Pool,
                    ),
                ) if decode_reps > 1 else nullcontext(),
            ):
                # ---- embed gather -> transpose -> hT [D, positions] ----
                for t in range(NT):
                    g = gpool.tile([128, D], f32)
                    nc.gpsimd.indirect_dma_start(
                        out=g[:],
                        out_offset=None,
                        in_=d_emb[:, :],
                        in_offset=bass.IndirectOffsetOnAxis(ap=it[:, t : t + 1], axis=0),
                    )
                    pt = tpsum.tile([128, 128], f32)
                    nc.tensor.transpose(pt[:], g[:], ident_sb[:])
                    nc.vector.tensor_copy(r(hT[:, t * 128 : (t + 1) * 128]), pt[:])

                # ---- mass layer: mT = softplus(wmT.T @ hT + b_mass) + EPS ----
                # softplus(z) = ln(exp(z) + 1)
                def mass_chunk(off, n):
                    pm = mpsum.tile([128, 512], f32, name="ps")
                    nc.tensor.matmul(
                        pm[:, :n], lhsT=wm_sb[:], rhs=r(hT[:, off : off + n]),
                        start=True, stop=True,
                    )
                    nc.scalar.activation(pm[:, :n], pm[:, :n], AF.Exp, bias=bm_sb[:, :1])
                    nc.scalar.activation(r(mT[:, off : off + n]), pm[:, :n], AF.Ln, bias=1.0)
                    nc.vector.tensor_scalar_add(
                        r(mT[:, off : off + n]), mT[:, off : off + n], EPS
                    )

                ve = nc.vector if stencil_eng == "vector" else nc.gpsimd

                def stencil_gen(q):
                    """K flux steps for quarter q, yielded in ~engine-balanced
                    units so decode tiles can be interleaved between them."""
                    mq, fq, dmq = mqs[q % 2], fqs[q % 2], dmqs[q % 2]
                    ve.tensor_copy(r(mq[:]), mT[:, QOWN * q : QOWN * q + QWP])
                    yield
                    for k in range(K):
                        p1 = mpsum.tile([128, 512], f32, name="ps")
                        nc.tensor.matmul(
                            p1[:, :QW], lhsT=wl_sb[:], rhs=r(mq[:, 0:QW]),
                            start=True, stop=False,
                        )
                        nc.tensor.matmul(
                            p1[:, :QW], lhsT=wr_sb[:], rhs=r(mq[:, 1 : 1 + QW]),
                            start=False, stop=True,
                        )
                        yield
                        # tanh(z) = 1 - 2/(exp(2z) + 1); b_f1 pre-doubled host-side
                        nc.scalar.activation(
                            p1[:, :QW], p1[:, :QW], AF.Exp, bias=b1_sb[:, :1], scale=2.0
                        )
                        yield
                        nc.vector.tensor_scalar_add(p1[:, :QW], p1[:, :QW], 1.0)
                        hid = hidp.tile([128, QW], f32)
                        with nc.allow_low_precision(reason="f32r is fp32-width"):
                            nc.vector.reciprocal(r(hid[:]), p1[:, :QW])
                        yield
                        ve.tensor_scalar(
                            r(hid[:]), hid[:], -2.0, 1.0,
                            op0=mybir.AluOpType.mult, op1=mybir.AluOpType.add,
                        )
                        yield
                        p2 = mpsum.tile([128, 512], f32, name="ps")
                        nc.tensor.matmul(
                            p2[:, :QW], lhsT=w2_sb[:], rhs=r(hid[:]),
                            start=True, stop=True,
                        )
                        yield
                        nc.scalar.activation(p2[:, :QW], p2[:, :QW], AF.Exp, bias=b2_sb[:, :1])
                        yield
                        nc.scalar.activation(fq[:, 1 : 1 + QW], p2[:, :QW], AF.Ln, bias=1.0)
                        yield
                        ve.tensor_mul(
                            fq[:, 1 : 1 + QW], fq[:, 1 : 1 + QW],
                            mask_sb[:, q * QW : (q + 1) * QW],
                        )
                        yield
                        ve.tensor_sub(dmq[:], fq[:, 0:QW], fq[:, 1 : 1 + QW])
                        yield
                        ve.tensor_add(r(mq[:, 0:QW]), mq[:, 0:QW], dmq[:])
                        if k < K - 1:
                            ve.tensor_scalar_max(r(mq[:, 0:QW]), mq[:, 0:QW], EPS)
                        else:
                            with nc.allow_low_precision(reason="bf16 decode operand"):
                                ve.tensor_scalar_max(
                                    m16[:, q * QOWN : (q + 1) * QOWN],
                                    mq[:, HALO : HALO + QOWN], EPS,
                                )
                        yield

                mass_chunk(*MASS_CHUNKS[0])
                for _ in stencil_gen(0):      # quarter 0: serial, critical path
                    pass
                mass_chunk(*MASS_CHUNKS[1])
                mass_chunk(*MASS_CHUNKS[2])
                if not interleave:
                    for q in range(1, NQ):
                        for _ in stencil_gen(q):
                            pass

                # ---- decode: out[r, v] = m_rows @ w_dec.T + b_dec ----
                # quarter-major; stencil for quarter q+1 interleaves 1 unit
                # per vocab block under quarter q's decode stream.
                i = 0
                for q in range(NQ):
                    gen = stencil_gen(q + 1) if (interleave and q + 1 < NQ) else None
                    for v0 in range(0, NVB, grp):
                        for rt in (2 * q, 2 * q + 1):
                            m_blk = m16[:, rt * 128 : (rt + 1) * 128]
                            st = stage.tile([128, grp * VBLK], f32)
                            for j in range(grp):
                                v = v0 + j
                                vs = slice(v * VBLK, (v + 1) * VBLK)
                                pd = dpsum.tile([128, VBLK], f32)
                                nc.tensor.matmul(
                                    pd[:], lhsT=ones_sb[:1, :], rhs=bdec_sb[:1, vs],
                                    start=True, stop=False,
                                )
                                nc.tensor.matmul(
                                    pd[:], lhsT=m_blk, rhs=wdec_sb[:, vs],
                                    start=False, stop=True,
                                )
                                ssl = st[:, j * VBLK : (j + 1) * VBLK]
                                if (i + j) % 2 == 0:
                                    nc.scalar.copy(ssl, pd[:])
                                else:
                                    nc.vector.tensor_copy(ssl, pd[:])
                            dma_eng = getattr(nc, rings[i % len(rings)])
                            dma_eng.dma_start(
                                d_out[rt * 128 : (rt + 1) * 128,
                                      v0 * VBLK : (v0 + grp) * VBLK],
                                st[:],
                            )
                            i += 1
                        if gen is not None:
                            next(gen, None)
                    if gen is not None:
                        for _ in gen:
                            pass

    # Keep every ACT function in one table load (Exp, Ln, Copy all live in
    # 'natural_log_exp_and_others'); blank other tables so the table-load
    # insertion pass can't thrash LoadActFuncSet (~1.3us each).
    import concourse.bacc as bacc_mod
    orig_get_tables = bacc_mod.get_activation_tables

    def only_ln_exp(arch):
        tabs = orig_get_tables(arch)
        return {
            k: (v if k == "natural_log_exp_and_others" else set())
            for k, v in tabs.items()
        }

    bacc_mod.get_activation_tables = only_ln_exp
    try:
        nc.compile()
    finally:
        bacc_mod.get_activation_tables = orig_get_tables
    return nc


def _prep_inputs(inputs):
    """Host-side shard prep: per-core input dict list."""
    import ml_dtypes

    bf16 = ml_dtypes.bfloat16

    x = np.asarray(inputs["x"]).astype(np.int32)            # [B, S]
    emb = np.ascontiguousarray(np.asarray(inputs["emb"], np.float32))
    w_mass = np.asarray(inputs["w_mass"], np.float32)
    b_mass = np.asarray(inputs["b_mass"], np.float32)
    w_f1 = np.asarray(inputs["w_f1"], np.float32)
    b_f1 = np.asarray(inputs["b_f1"], np.float32)
    w_f2 = np.asarray(inputs["w_f2"], np.float32)
    b_f2 = np.asarray(inputs["b_f2"], np.float32)
    cfl = float(np.asarray(inputs["cfl_raw"]))
    w_dec = np.asarray(inputs["w_dec"], np.float32)
    b_dec = np.asarray(inputs["b_dec"], np.float32)

    dt = float(1.0 / (1.0 + np.exp(-cfl)))

    common = {
        "emb": emb,
        "wmT": np.ascontiguousarray(w_mass.T),
        "wf1lT": np.ascontiguousarray(w_f1[:, :D].T),
        "wf1rT": np.ascontiguousarray(w_f1[:, D:].T),
        "wf2T": np.ascontiguousarray(w_f2.T),
        "b_mass": np.ascontiguousarray(b_mass[:, None]),
        # device computes tanh(z+b) as 1 - 2/(exp(2z + 2b) + 1) with scale=2 on z
        "b_f1": np.ascontiguousarray((2.0 * b_f1)[:, None]),
        "b_f2": np.ascontiguousarray(b_f2[:, None]),
        "ones16": np.ones((1, D), bf16),
        "bdec16": np.ascontiguousarray(b_dec.astype(bf16)[None, :]),
        "wdec16T": np.ascontiguousarray(w_dec.T.astype(bf16)),
        "ident": np.eye(D, dtype=np.float32),
    }

    in_maps = []
    for c in range(NCORES):
        b, half = divmod(c, 2)
        idx = np.zeros(LPAD, np.int32)
        if half == 0:
            idx[HALO : HALO + (L - HALO)] = x[b, 0 : L - HALO]
        else:
            idx[0 : L - HALO] = x[b, S - (L - HALO) : S]
        # per-quarter edge masks: dt everywhere; fake edge (QW-1) always 0;
        # true-boundary sides zero the outer 3 edges (halo-overlap trick
        # handles interior quarter boundaries with no masking).
        maskq = np.full((NQ, QW), dt, np.float32)
        maskq[:, QW - 1] = 0.0
        if half == 0:
            maskq[0, 0:HALO] = 0.0
        else:
            maskq[NQ - 1, QW - 1 - HALO : QW - 1] = 0.0
        m = dict(common)
        m["idx"] = np.ascontiguousarray(idx.reshape(NT, 128).T)     # [128, NT]
        m["maskq"] = np.ascontiguousarray(
            np.broadcast_to(maskq.reshape(1, NQ * QW), (D, NQ * QW)).astype(np.float32)
        )
        in_maps.append(m)
    return in_maps


def get_program():
    if "nc" not in _CACHE:
        _CACHE["nc"] = build_program()
    return _CACHE["nc"]


def run(inputs, trace=False, **kw):
    """Returns (full_output [B,S,V] float32, BassKernelResults)."""
    from concourse.bass_utils import run_bass_kernel_spmd

    nc = get_program()
    in_maps = _prep_inputs(inputs)
    res = run_bass_kernel_spmd(
        nc, in_maps, core_ids=list(range(NCORES)), trace=trace, **kw
    )
    full = np.empty((B * S, V), np.float32)
    for c in range(NCORES):
        full[c * LOWN : (c + 1) * LOWN] = res.results[c]["out"]
    return full.reshape(B, S, V), res


def kernel(**inputs):
    out, _ = run(inputs, trace=False)
    return out
